# revision 16
# baseline (speedup 1.0000x reference)
"""Trainium2 Bass kernel for nn_BSHConv3D: spherical-harmonic 3^3 conv.

The whole module collapses to one dense 3D convolution
x[1,48,48,48,8] -> out[48,48,48, 512] with combined weights
W[3,3,3, 8, 512] (the central 1x1x1 conv folds into the center tap, the
bias rides on an extra constant-ones contraction row).

Per-core layout (D sharded 8 x 6 slabs, halo 1):
  - host pre-pads each core's x slab (plus halos) to [8ch + ones, 8*50*50]
  - on-chip: 9-way (kh,kw)-shifted im2col S[73, ~20k] built with
    SBUF->SBUF DMAs; kd handled as 3 PSUM-accumulating fp32r matmuls with
    free-dim offsets of +-2500
  - matmul: lhsT = S[:, zb+(kd-1)*2500 :][73 x 128], rhs = Wc[kd][73 x 512]
    -> PSUM [128 pos, 512 ch]
  - PSUM evacuated by VectorE/ScalarE alternating, valid rows DMA'd to HBM
"""

from contextlib import ExitStack

import numpy as np

import concourse.bass as bass
from concourse import bacc
import concourse.mybir as mybir
import concourse.tile as tile
from concourse.bass_utils import run_bass_kernel_spmd

B, D, H, W, C = 1, 48, 48, 48, 8
KS, R, DEG, NH, OUT = 3, 2, 3, 16, 16
NCORES = 8
DL = D // NCORES  # 6 output slabs per core
HP = WP = 50  # zero-padded H/W
SLAB = HP * WP  # 2500
NSLAB = DL + 2  # local slabs incl. halos
MARGIN = 64  # left margin in the z buffer (shift slack)
UD = NSLAB * SLAB  # 20000 payload columns
SZ = 20352  # total z columns per row
NCH = OUT * NH * 2  # 512 output channels (f, n, re/im)
KC = 73  # contraction rows: 9 taps x 8 ch + ones row
ZB0 = MARGIN + SLAB  # first computed z column
TM = 128  # positions per matmul tile
NT = 117  # z tiles per core
NVALID = DL * H * W  # 13824 valid output rows per core
NPAD = NT * TM  # 14976 padded output rows actually written
BUILD_LO = MARGIN
BUILD_HI = MARGIN + 20160  # build S over [BUILD_LO, BUILD_HI), 40 blocks of 504
ZBLK = 504  # z columns per interleaved DRAM block (~2KB descriptors)
NZ_CHUNKS = 4  # im2col load chunking so matmuls can start early
GT = 9  # z tiles grouped per output DMA (117 = 13 groups of 9)

# module-level knobs for the test harness (graders just call kernel())
TRACE = False
LAST_RESULTS = None


def _valid_row_index():
    """Indices into the padded [NPAD] output rows that are real outputs,
    in output raster order."""
    u = np.arange(NPAD) + (ZB0 - MARGIN)
    dl = u // SLAB
    hp = (u % SLAB) // WP
    wp = u % WP
    mask = (dl >= 1) & (dl < 1 + DL) & (hp >= 1) & (hp <= H) & (wp >= 1) & (wp <= W)
    idx = np.nonzero(mask)[0]
    assert idx.size == NVALID, idx.size
    return idx


_VALID_IDX = _valid_row_index()


def _build_program():
    f32 = mybir.dt.float32
    f32r = mybir.dt.float32r
    nc = bacc.Bacc("TRN2", debug=False)
    # DRAM layouts are block-interleaved so every DMA descriptor is a
    # non-mergeable ~2KB run: descriptors <=2KB spread across all 16 SDMA
    # engines (~235GB/s); bigger merged descriptors pin to ONE engine
    # (~26GB/s measured).
    nzb = (BUILD_HI - BUILD_LO) // ZBLK  # 40 z blocks
    xin = nc.dram_tensor("xin", [nzb, KC, ZBLK], f32r, kind="ExternalInput").ap()
    wc = nc.dram_tensor("wc", [3, KC, NCH], f32r, kind="ExternalInput").ap()
    out = nc.dram_tensor("out", [NPAD, NCH], f32, kind="ExternalOutput").ap()

    with tile.TileContext(nc) as tc, ExitStack() as ctx:
        const_pool = ctx.enter_context(tc.tile_pool(name="const", bufs=1))
        stage_pool = ctx.enter_context(tc.tile_pool(name="stage", bufs=2))
        psum_pool = ctx.enter_context(tc.tile_pool(name="psum", bufs=6, space="PSUM"))

        S = const_pool.tile([KC, SZ], f32r, name="S")
        Wt = const_pool.tile([KC, 3 * NCH], f32r, name="Wt")

        nc.sync.dma_start(
            Wt[:, :].rearrange("p (kd c) -> p kd c", kd=3),
            wc.rearrange("kd p c -> p kd c"),
        )

        # im2col is pre-built host-side; just load it, chunked in z so the
        # first matmuls can start early
        bpc = nzb // NZ_CHUNKS
        for zc in range(NZ_CHUNKS):
            lo = BUILD_LO + zc * bpc * ZBLK
            hi = lo + bpc * ZBLK
            src = xin[zc * bpc : (zc + 1) * bpc].rearrange("zb p c -> p zb c")
            dst = S[:, lo:hi].rearrange("p (zb c) -> p zb c", zb=bpc)
            nc.sync.dma_start(dst, src)

        for g0 in range(0, NT, GT):
            st = stage_pool.tile([TM, GT * NCH], f32, name="st")
            for g in range(GT):
                t = g0 + g
                zb = ZB0 + t * TM
                ps = psum_pool.tile([TM, NCH], f32, name="ps")
                for kd in range(3):
                    a = zb + (kd - 1) * SLAB
                    nc.tensor.matmul(
                        ps[:, :],
                        S[0:KC, a : a + TM],
                        Wt[0:KC, kd * NCH : (kd + 1) * NCH],
                        start=(kd == 0),
                        stop=(kd == 2),
                    )
                dst = st[:, g * NCH : (g + 1) * NCH]
                if t % 2 == 0:
                    nc.vector.tensor_copy(dst, ps[:, :])
                else:
                    nc.scalar.copy(dst, ps[:, :])
            # one DMA per group: SBUF [p, (g c)] -> DRAM rows [(g p), c],
            # iterated (p, g, c) so the SBUF partition dim stays dim 0
            src3 = st[:, :].rearrange("p (g c) -> p g c", g=GT)
            dst3 = out[g0 * TM : (g0 + GT) * TM, :].rearrange(
                "(g p) c -> p g c", p=TM
            )
            nc.sync.dma_start(dst3, src3)
    nc.compile()
    return nc


_program_cache = {}


def _get_program():
    if "nc" not in _program_cache:
        _program_cache["nc"] = _build_program()
    return _program_cache["nc"]


def _host_weights(atoms_real, atoms_imag, w, w_center, b_center):
    idx = np.repeat(np.arange(DEG + 1), [2 * n + 1 for n in range(DEG + 1)])
    w_exp = w[..., idx]  # [C,F,R,NH]
    WR = np.einsum("dhwrn,cfrn->dhwcfn", atoms_real, w_exp)
    WI = np.einsum("dhwrn,cfrn->dhwcfn", atoms_imag, w_exp)
    Wfull = np.stack([WR, WI], axis=-1)  # [3,3,3,C,F,NH,2]
    Wc = np.zeros((3, KC, NCH), np.float32)
    Wc[:, :72, :] = Wfull.reshape(3, 72, NCH)
    Wc[1, 32:40, 0::32] += w_center  # central 1x1x1 conv onto (f, n=0, re)
    Wc[1, 72, 0::32] = b_center
    return Wc


def kernel(x, atoms_real, atoms_imag, w, w_center, b_center):
    global LAST_RESULTS
    x = np.asarray(x, np.float32)
    Wc = _host_weights(
        np.asarray(atoms_real, np.float32),
        np.asarray(atoms_imag, np.float32),
        np.asarray(w, np.float32),
        np.asarray(w_center, np.float32),
        np.asarray(b_center, np.float32),
    )

    xt = np.transpose(x[0], (3, 0, 1, 2))  # [C,D,H,W]
    xpad = np.zeros((C, D + 2, HP, WP), np.float32)
    xpad[:, 1 : D + 1, 1 : H + 1, 1 : W + 1] = xt

    n_build = BUILD_HI - BUILD_LO
    nzb = n_build // ZBLK
    in_maps = []
    for core in range(NCORES):
        d0 = core * DL
        pbuf = np.zeros((C, SZ), np.float32)
        pbuf[:, MARGIN : MARGIN + UD] = xpad[:, d0 : d0 + NSLAB].reshape(C, UD)
        buf = np.zeros((KC, SZ), np.float32)
        for kh in range(3):
            for kw in range(3):
                off = (kh - 1) * WP + (kw - 1)
                r0 = (kh * 3 + kw) * 8
                buf[r0 : r0 + 8, BUILD_LO:BUILD_HI] = pbuf[
                    :, BUILD_LO + off : BUILD_LO + off + n_build
                ]
        buf[72, :] = 1.0
        # block-interleave: [nzb, KC, ZBLK]
        xin3 = np.ascontiguousarray(
            buf[:, BUILD_LO:BUILD_HI].reshape(KC, nzb, ZBLK).swapaxes(0, 1)
        )
        in_maps.append({"xin": xin3, "wc": Wc})

    nc = _get_program()
    res = run_bass_kernel_spmd(
        nc, in_maps, core_ids=list(range(NCORES)), trace=TRACE
    )
    LAST_RESULTS = res
    outs = [res.results[i]["out"][_VALID_IDX] for i in range(NCORES)]
    full = np.concatenate([o.reshape(DL, H, W, OUT, NH, 2) for o in outs], axis=0)
    return full[None]


# revision 19
# speedup vs baseline: 1.9059x; 1.9059x over previous
"""Trainium2 Bass kernel for nn_BSHConv3D: spherical-harmonic 3^3 conv.

The whole module collapses to one dense 3D convolution
x[1,48,48,48,8] -> out[48,48,48, 512] with combined weights
W[3,3,3, 8, 512] (the central 1x1x1 conv folds into the center tap, the
bias rides on an extra constant-ones contraction row).

Per-core layout (D sharded 8 x 6 slabs, halo 1):
  - host pre-pads each core's x slab (plus halos) to [8ch + ones, 8*50*50]
  - on-chip: 9-way (kh,kw)-shifted im2col S[73, ~20k] built with
    SBUF->SBUF DMAs; kd handled as 3 PSUM-accumulating fp32r matmuls with
    free-dim offsets of +-2500
  - matmul: lhsT = S[:, zb+(kd-1)*2500 :][73 x 128], rhs = Wc[kd][73 x 512]
    -> PSUM [128 pos, 512 ch]
  - PSUM evacuated by VectorE/ScalarE alternating, valid rows DMA'd to HBM
"""

from contextlib import ExitStack

import ml_dtypes
import numpy as np

import concourse.bass as bass
from concourse import bacc
import concourse.mybir as mybir
import concourse.tile as tile
from concourse.bass_utils import run_bass_kernel_spmd

B, D, H, W, C = 1, 48, 48, 48, 8
KS, R, DEG, NH, OUT = 3, 2, 3, 16, 16
NCORES = 8
DL = D // NCORES  # 6 output slabs per core
HP = WP = 50  # zero-padded H/W
SLAB = HP * WP  # 2500
NSLAB = DL + 2  # local slabs incl. halos
MARGIN = 64  # left margin in the z buffer (shift slack)
UD = NSLAB * SLAB  # 20000 payload columns
SZ = 20352  # total z columns per row
NCH = OUT * NH * 2  # 512 output channels (f, n, re/im)
KC = 73  # contraction rows: 9 taps x 8 ch + ones row
ZB0 = MARGIN + SLAB  # first computed z column
TM = 128  # positions per matmul tile
NT = 117  # z tiles per core
NVALID = DL * H * W  # 13824 valid output rows per core
NPAD = NT * TM  # 14976 padded output rows actually written
BUILD_LO = MARGIN
BUILD_HI = MARGIN + 20160  # build S over [BUILD_LO, BUILD_HI), 40 blocks of 504
ZBLK = 504  # z columns per interleaved DRAM block (~2KB descriptors)
NZ_CHUNKS = 4  # im2col load chunking so matmuls can start early
GT = 9  # z tiles grouped per output DMA (117 = 13 groups of 9)

# module-level knobs for the test harness (graders just call kernel())
TRACE = False
LAST_RESULTS = None


def _valid_row_index():
    """Indices into the padded [NPAD] output rows that are real outputs,
    in output raster order."""
    u = np.arange(NPAD) + (ZB0 - MARGIN)
    dl = u // SLAB
    hp = (u % SLAB) // WP
    wp = u % WP
    mask = (dl >= 1) & (dl < 1 + DL) & (hp >= 1) & (hp <= H) & (wp >= 1) & (wp <= W)
    idx = np.nonzero(mask)[0]
    assert idx.size == NVALID, idx.size
    return idx


_VALID_IDX = _valid_row_index()


def _build_program():
    f32 = mybir.dt.float32
    f32r = mybir.dt.float32r
    bf16 = mybir.dt.bfloat16
    nc = bacc.Bacc("TRN2", debug=False)
    xin = nc.dram_tensor("xin", [KC, SZ], bf16, kind="ExternalInput").ap()
    wc = nc.dram_tensor("wc", [3, KC, NCH], bf16, kind="ExternalInput").ap()
    out = nc.dram_tensor("out", [NPAD, NCH], f32, kind="ExternalOutput").ap()

    with tile.TileContext(nc) as tc, ExitStack() as ctx:
        const_pool = ctx.enter_context(tc.tile_pool(name="const", bufs=1))
        stage_pool = ctx.enter_context(tc.tile_pool(name="stage", bufs=2))
        psum_pool = ctx.enter_context(tc.tile_pool(name="psum", bufs=6, space="PSUM"))

        S = const_pool.tile([KC, SZ], bf16, name="S")
        Wt = const_pool.tile([KC, 3 * NCH], bf16, name="Wt")

        # all HBM->SBUF loads ride SWDGE (gpsimd): the HWDGE path pins each
        # load to a single SDMA engine (~26GB/s); SWDGE spreads descriptors
        # across all 16 (~340GB/s)
        nc.gpsimd.dma_start(
            Wt[:, :].rearrange("p (kd c) -> p kd c", kd=3),
            wc.rearrange("kd p c -> p kd c"),
        )

        # im2col is pre-built host-side; just load it, chunked in z so the
        # first matmuls can start early
        nzc = (BUILD_HI - BUILD_LO) // NZ_CHUNKS
        for zc in range(NZ_CHUNKS):
            lo = BUILD_LO + zc * nzc
            hi = lo + nzc
            nc.gpsimd.dma_start(S[:, lo:hi], xin[:, lo:hi])

        for g0 in range(0, NT, GT):
            st = stage_pool.tile([TM, GT * NCH], f32, name="st")
            for g in range(GT):
                t = g0 + g
                zb = ZB0 + t * TM
                ps = psum_pool.tile([TM, NCH], f32, name="ps")
                for kd in range(3):
                    a = zb + (kd - 1) * SLAB
                    nc.tensor.matmul(
                        ps[:, :],
                        S[0:KC, a : a + TM],
                        Wt[0:KC, kd * NCH : (kd + 1) * NCH],
                        start=(kd == 0),
                        stop=(kd == 2),
                    )
                dst = st[:, g * NCH : (g + 1) * NCH]
                if t % 2 == 0:
                    nc.vector.tensor_copy(dst, ps[:, :])
                else:
                    nc.scalar.copy(dst, ps[:, :])
            # one DMA per group: SBUF [p, (g c)] -> DRAM rows [(g p), c],
            # iterated (p, g, c) so the SBUF partition dim stays dim 0
            src3 = st[:, :].rearrange("p (g c) -> p g c", g=GT)
            dst3 = out[g0 * TM : (g0 + GT) * TM, :].rearrange(
                "(g p) c -> p g c", p=TM
            )
            nc.sync.dma_start(dst3, src3)
    nc.compile()
    return nc


_program_cache = {}


def _get_program():
    if "nc" not in _program_cache:
        _program_cache["nc"] = _build_program()
    return _program_cache["nc"]


def _host_weights(atoms_real, atoms_imag, w, w_center, b_center):
    idx = np.repeat(np.arange(DEG + 1), [2 * n + 1 for n in range(DEG + 1)])
    w_exp = w[..., idx]  # [C,F,R,NH]
    WR = np.einsum("dhwrn,cfrn->dhwcfn", atoms_real, w_exp)
    WI = np.einsum("dhwrn,cfrn->dhwcfn", atoms_imag, w_exp)
    Wfull = np.stack([WR, WI], axis=-1)  # [3,3,3,C,F,NH,2]
    Wc = np.zeros((3, KC, NCH), np.float32)
    Wc[:, :72, :] = Wfull.reshape(3, 72, NCH)
    Wc[1, 32:40, 0::32] += w_center  # central 1x1x1 conv onto (f, n=0, re)
    Wc[1, 72, 0::32] = b_center
    return Wc


def kernel(x, atoms_real, atoms_imag, w, w_center, b_center):
    global LAST_RESULTS
    x = np.asarray(x, np.float32)
    Wc = _host_weights(
        np.asarray(atoms_real, np.float32),
        np.asarray(atoms_imag, np.float32),
        np.asarray(w, np.float32),
        np.asarray(w_center, np.float32),
        np.asarray(b_center, np.float32),
    )

    Wc16 = Wc.astype(ml_dtypes.bfloat16)
    xt = np.transpose(x[0], (3, 0, 1, 2))  # [C,D,H,W]
    xpad = np.zeros((C, D + 2, HP, WP), np.float32)
    xpad[:, 1 : D + 1, 1 : H + 1, 1 : W + 1] = xt

    n_build = BUILD_HI - BUILD_LO
    nzb = n_build // ZBLK
    in_maps = []
    for core in range(NCORES):
        d0 = core * DL
        pbuf = np.zeros((C, SZ), np.float32)
        pbuf[:, MARGIN : MARGIN + UD] = xpad[:, d0 : d0 + NSLAB].reshape(C, UD)
        buf = np.zeros((KC, SZ), np.float32)
        for kh in range(3):
            for kw in range(3):
                off = (kh - 1) * WP + (kw - 1)
                r0 = (kh * 3 + kw) * 8
                buf[r0 : r0 + 8, BUILD_LO:BUILD_HI] = pbuf[
                    :, BUILD_LO + off : BUILD_LO + off + n_build
                ]
        buf[72, :] = 1.0
        in_maps.append({"xin": buf.astype(ml_dtypes.bfloat16), "wc": Wc16})

    nc = _get_program()
    res = run_bass_kernel_spmd(
        nc, in_maps, core_ids=list(range(NCORES)), trace=TRACE
    )
    LAST_RESULTS = res
    outs = [res.results[i]["out"][_VALID_IDX] for i in range(NCORES)]
    full = np.concatenate([o.reshape(DL, H, W, OUT, NH, 2) for o in outs], axis=0)
    return full[None]


# revision 20
# speedup vs baseline: 2.1773x; 1.1424x over previous
"""Trainium2 Bass kernel for nn_BSHConv3D: spherical-harmonic 3^3 conv.

The whole module collapses to one dense 3D convolution
x[1,48,48,48,8] -> out[48,48,48, 512] with combined weights
W[3,3,3, 8, 512] (the central 1x1x1 conv folds into the center tap, the
bias rides on an extra constant-ones contraction row).

Per-core (D sharded 8 x 6 slabs, halo 1):
  - host builds the FULL 27-tap im2col: S[217, 14976] where row
    (kd,kh,kw,c) is the correspondingly shifted padded x volume and row
    216 is constant ones; z = flattened (d,h,w) padded coords
  - matmul per 128-position tile: 2 PSUM-accumulating matmuls
    (K = 128 + 89 contraction rows) x N=512 output channels
  - PE streams at 1 col / 1.2GHz-cycle here regardless of dtype, so
    2 matmuls/tile (1024 streamed cols) is the floor
  - PSUM evacuated by VectorE/ScalarE alternating into a 9-tile group
    staging buffer, one ~2.3MB output DMA per group (2KB descriptors
    spread across all 16 SDMA engines; bigger merged descriptors pin to
    one engine at ~26GB/s)
  - HBM->SBUF loads ride SWDGE (gpsimd) for the same reason
"""

from contextlib import ExitStack

import ml_dtypes
import numpy as np

import concourse.bass as bass
from concourse import bacc
import concourse.mybir as mybir
import concourse.tile as tile
from concourse.bass_utils import run_bass_kernel_spmd

B, D, H, W, C = 1, 48, 48, 48, 8
KS, R, DEG, NH, OUT = 3, 2, 3, 16, 16
NCORES = 8
DL = D // NCORES  # 6 output slabs per core
HP = WP = 50  # zero-padded H/W
SLAB = HP * WP  # 2500
NSLAB = DL + 2  # local slabs incl. halos
MARGIN = 64  # left margin in the host z buffer (shift slack)
UD = NSLAB * SLAB  # 20000 payload columns
SZ = 20352  # host-side padded z columns
NCH = OUT * NH * 2  # 512 output channels (f, n, re/im)
KC = 27 * C + 1  # 217 contraction rows: 27 taps x 8 ch + ones row
KA = 128  # contraction chunk A (SBUF partition limit)
KB = KC - KA  # 89
ZB0 = MARGIN + SLAB  # first computed z column (host coords)
TM = 128  # positions per matmul tile
NT = 117  # z tiles per core
NZ = NT * TM  # 14976 z columns materialized on chip
NVALID = DL * H * W  # 13824 valid output rows per core
NPAD = NZ  # padded output rows written
NZ_CHUNKS = 4  # im2col load chunking so matmuls can start early
GT = 9  # z tiles grouped per output DMA (117 = 13 groups of 9)

USE_BF16 = False  # False -> fp32r matmuls (rel err ~1.5e-4)

# module-level knobs for the test harness (graders just call kernel())
TRACE = False
LAST_RESULTS = None


def _valid_row_index():
    """Indices into the padded [NPAD] output rows that are real outputs,
    in output raster order."""
    u = np.arange(NPAD) + (ZB0 - MARGIN)
    dl = u // SLAB
    hp = (u % SLAB) // WP
    wp = u % WP
    mask = (dl >= 1) & (dl < 1 + DL) & (hp >= 1) & (hp <= H) & (wp >= 1) & (wp <= W)
    idx = np.nonzero(mask)[0]
    assert idx.size == NVALID, idx.size
    return idx


_VALID_IDX = _valid_row_index()


def _build_program():
    f32 = mybir.dt.float32
    mdt = mybir.dt.bfloat16 if USE_BF16 else mybir.dt.float32r
    nc = bacc.Bacc("TRN2", debug=False)
    xin = nc.dram_tensor("xin", [KC, NZ], mdt, kind="ExternalInput").ap()
    wc = nc.dram_tensor("wc", [KC, NCH], mdt, kind="ExternalInput").ap()
    out = nc.dram_tensor("out", [NPAD, NCH], f32, kind="ExternalOutput").ap()

    with tile.TileContext(nc) as tc, ExitStack() as ctx:
        const_pool = ctx.enter_context(tc.tile_pool(name="const", bufs=1))
        stage_pool = ctx.enter_context(tc.tile_pool(name="stage", bufs=2))
        psum_pool = ctx.enter_context(tc.tile_pool(name="psum", bufs=6, space="PSUM"))

        SA = const_pool.tile([KA, NZ], mdt, name="SA")
        SB = const_pool.tile([KB, NZ], mdt, name="SB")
        WtA = const_pool.tile([KA, NCH], mdt, name="WtA")
        WtB = const_pool.tile([KB, NCH], mdt, name="WtB")

        # all HBM->SBUF loads ride SWDGE (gpsimd): the HWDGE path pins a
        # load to a single SDMA engine; SWDGE spreads across all 16
        nc.gpsimd.dma_start(WtA[:, :], wc[0:KA])
        nc.gpsimd.dma_start(WtB[:, :], wc[KA:KC])

        nzc = NZ // NZ_CHUNKS
        for zc in range(NZ_CHUNKS):
            lo, hi = zc * nzc, (zc + 1) * nzc
            nc.gpsimd.dma_start(SA[:, lo:hi], xin[0:KA, lo:hi])
            nc.gpsimd.dma_start(SB[:, lo:hi], xin[KA:KC, lo:hi])

        for g0 in range(0, NT, GT):
            st = stage_pool.tile([TM, GT * NCH], f32, name="st")
            for g in range(GT):
                t = g0 + g
                zb = t * TM
                ps = psum_pool.tile([TM, NCH], f32, name="ps")
                nc.tensor.matmul(
                    ps[:, :], SA[:, zb : zb + TM], WtA[:, :],
                    start=True, stop=False,
                )
                nc.tensor.matmul(
                    ps[:, :], SB[:, zb : zb + TM], WtB[:, :],
                    start=False, stop=True,
                )
                dst = st[:, g * NCH : (g + 1) * NCH]
                if t % 2 == 0:
                    nc.vector.tensor_copy(dst, ps[:, :])
                else:
                    nc.scalar.copy(dst, ps[:, :])
            # one DMA per group: SBUF [p, (g c)] -> DRAM rows [(g p), c],
            # iterated (p, g, c) so the SBUF partition dim stays dim 0
            src3 = st[:, :].rearrange("p (g c) -> p g c", g=GT)
            dst3 = out[g0 * TM : (g0 + GT) * TM, :].rearrange(
                "(g p) c -> p g c", p=TM
            )
            nc.sync.dma_start(dst3, src3)
    nc.compile()
    return nc


_program_cache = {}


def _get_program():
    if "nc" not in _program_cache:
        _program_cache["nc"] = _build_program()
    return _program_cache["nc"]


def _host_weights(atoms_real, atoms_imag, w, w_center, b_center):
    idx = np.repeat(np.arange(DEG + 1), [2 * n + 1 for n in range(DEG + 1)])
    w_exp = w[..., idx]  # [C,F,R,NH]
    WR = np.einsum("dhwrn,cfrn->dhwcfn", atoms_real, w_exp)
    WI = np.einsum("dhwrn,cfrn->dhwcfn", atoms_imag, w_exp)
    Wfull = np.stack([WR, WI], axis=-1)  # [3,3,3,C,F,NH,2]
    Wc = np.zeros((KC, NCH), np.float32)
    Wc[: KC - 1, :] = Wfull.reshape(KC - 1, NCH)
    # central 1x1x1 conv onto (f, n=0, re): tap (kd=1,kh=1,kw=1) rows 104..111
    Wc[104:112, 0::32] += w_center
    Wc[KC - 1, 0::32] = b_center
    return Wc


def kernel(x, atoms_real, atoms_imag, w, w_center, b_center):
    global LAST_RESULTS
    x = np.asarray(x, np.float32)
    Wc = _host_weights(
        np.asarray(atoms_real, np.float32),
        np.asarray(atoms_imag, np.float32),
        np.asarray(w, np.float32),
        np.asarray(w_center, np.float32),
        np.asarray(b_center, np.float32),
    )
    hdt = ml_dtypes.bfloat16 if USE_BF16 else np.float32
    Wc = Wc.astype(hdt)

    xt = np.transpose(x[0], (3, 0, 1, 2))  # [C,D,H,W]
    xpad = np.zeros((C, D + 2, HP, WP), np.float32)
    xpad[:, 1 : D + 1, 1 : H + 1, 1 : W + 1] = xt

    in_maps = []
    for core in range(NCORES):
        d0 = core * DL
        pbuf = np.zeros((C, SZ), np.float32)
        pbuf[:, MARGIN : MARGIN + UD] = xpad[:, d0 : d0 + NSLAB].reshape(C, UD)
        buf = np.empty((KC, NZ), np.float32)
        r = 0
        for kd in range(3):
            for kh in range(3):
                for kw in range(3):
                    off = (kd - 1) * SLAB + (kh - 1) * WP + (kw - 1)
                    buf[r : r + 8] = pbuf[:, ZB0 + off : ZB0 + off + NZ]
                    r += 8
        buf[KC - 1] = 1.0
        in_maps.append({"xin": buf.astype(hdt), "wc": Wc})

    nc = _get_program()
    res = run_bass_kernel_spmd(
        nc, in_maps, core_ids=list(range(NCORES)), trace=TRACE
    )
    LAST_RESULTS = res
    outs = [res.results[i]["out"][_VALID_IDX] for i in range(NCORES)]
    full = np.concatenate([o.reshape(DL, H, W, OUT, NH, 2) for o in outs], axis=0)
    return full[None]


# revision 21
# speedup vs baseline: 3.2982x; 1.5148x over previous
"""Trainium2 Bass kernel for nn_BSHConv3D: spherical-harmonic 3^3 conv.

The whole module collapses to one dense 3D convolution
x[1,48,48,48,8] -> out[48,48,48, 512] with combined weights
W[3,3,3, 8, 512] (the central 1x1x1 conv folds into the center tap, the
bias rides on an extra constant-ones contraction row).

Per-core (D sharded 8 x 6 slabs, halo 1):
  - host builds the FULL 27-tap im2col: S[217, 14976] where row
    (kd,kh,kw,c) is the correspondingly shifted padded x volume and row
    216 is constant ones; z = flattened (d,h,w) padded coords
  - matmul per 128-position tile: 2 PSUM-accumulating matmuls
    (K = 128 + 89 contraction rows) x N=512 output channels
  - PE streams at 1 col / 1.2GHz-cycle here regardless of dtype, so
    2 matmuls/tile (1024 streamed cols) is the floor
  - PSUM evacuated by VectorE/ScalarE alternating into a 9-tile group
    staging buffer, one ~2.3MB output DMA per group (2KB descriptors
    spread across all 16 SDMA engines; bigger merged descriptors pin to
    one engine at ~26GB/s)
  - HBM->SBUF loads ride SWDGE (gpsimd) for the same reason
"""

from contextlib import ExitStack

import ml_dtypes
import numpy as np

import concourse.bass as bass
from concourse import bacc
import concourse.mybir as mybir
import concourse.tile as tile
from concourse.bass_utils import run_bass_kernel_spmd

B, D, H, W, C = 1, 48, 48, 48, 8
KS, R, DEG, NH, OUT = 3, 2, 3, 16, 16
NCORES = 8
DL = D // NCORES  # 6 output slabs per core
HP = WP = 50  # zero-padded H/W
SLAB = HP * WP  # 2500
NSLAB = DL + 2  # local slabs incl. halos
MARGIN = 64  # left margin in the host z buffer (shift slack)
UD = NSLAB * SLAB  # 20000 payload columns
SZ = 20352  # host-side padded z columns
NCH = OUT * NH * 2  # 512 output channels (f, n, re/im)
KC = 27 * C + 1  # 217 contraction rows: 27 taps x 8 ch + ones row
KA = 128  # contraction chunk A (SBUF partition limit)
KB = KC - KA  # 89
ZB0 = MARGIN + SLAB  # first computed z column (host coords)
TM = 128  # positions per matmul tile
NT = 117  # z tiles per core
NZ = NT * TM  # 14976 z columns materialized on chip
NVALID = DL * H * W  # 13824 valid output rows per core
NPAD = NZ  # padded output rows written
NZ_CHUNKS = 6  # im2col load chunking so matmuls can start early
GT = 9  # z tiles grouped per output DMA (117 = 13 groups of 9)

IO_DTYPE = "fp16"  # "fp16" | "bf16" | "f32r" matmul input dtype
OUT_DTYPE = "fp16"  # "fp16" | "f32" output DMA dtype (host upcasts)

# module-level knobs for the test harness (graders just call kernel())
TRACE = False
LAST_RESULTS = None


def _valid_row_index():
    """Indices into the padded [NPAD] output rows that are real outputs,
    in output raster order."""
    u = np.arange(NPAD) + (ZB0 - MARGIN)
    dl = u // SLAB
    hp = (u % SLAB) // WP
    wp = u % WP
    mask = (dl >= 1) & (dl < 1 + DL) & (hp >= 1) & (hp <= H) & (wp >= 1) & (wp <= W)
    idx = np.nonzero(mask)[0]
    assert idx.size == NVALID, idx.size
    return idx


_VALID_IDX = _valid_row_index()


_MDT = {"fp16": mybir.dt.float16, "bf16": mybir.dt.bfloat16, "f32r": mybir.dt.float32r}


def _build_program():
    f32 = mybir.dt.float32
    mdt = _MDT[IO_DTYPE]
    odt = mybir.dt.float16 if OUT_DTYPE == "fp16" else f32
    nc = bacc.Bacc("TRN2", debug=False)
    xin = nc.dram_tensor("xin", [KC, NZ], mdt, kind="ExternalInput").ap()
    wc = nc.dram_tensor("wc", [KC, NCH], mdt, kind="ExternalInput").ap()
    out = nc.dram_tensor("out", [NPAD, NCH], odt, kind="ExternalOutput").ap()

    with tile.TileContext(nc) as tc, ExitStack() as ctx:
        const_pool = ctx.enter_context(tc.tile_pool(name="const", bufs=1))
        stage_pool = ctx.enter_context(tc.tile_pool(name="stage", bufs=2))
        psum_pool = ctx.enter_context(tc.tile_pool(name="psum", bufs=6, space="PSUM"))

        SA = const_pool.tile([KA, NZ], mdt, name="SA")
        SB = const_pool.tile([KB, NZ], mdt, name="SB")
        WtA = const_pool.tile([KA, NCH], mdt, name="WtA")
        WtB = const_pool.tile([KB, NCH], mdt, name="WtB")

        # all HBM->SBUF loads ride SWDGE (gpsimd): the HWDGE path pins a
        # load to a single SDMA engine; SWDGE spreads across all 16
        nc.gpsimd.dma_start(WtA[:, :], wc[0:KA])
        nc.gpsimd.dma_start(WtB[:, :], wc[KA:KC])

        nzc = NZ // NZ_CHUNKS
        for zc in range(NZ_CHUNKS):
            lo, hi = zc * nzc, (zc + 1) * nzc
            nc.gpsimd.dma_start(SA[:, lo:hi], xin[0:KA, lo:hi])
            nc.gpsimd.dma_start(SB[:, lo:hi], xin[KA:KC, lo:hi])
        del f32  # staging dtype below follows the output dtype

        for g0 in range(0, NT, GT):
            st = stage_pool.tile([TM, GT * NCH], odt, name="st")
            for g in range(GT):
                t = g0 + g
                zb = t * TM
                ps = psum_pool.tile([TM, NCH], mybir.dt.float32, name="ps")
                nc.tensor.matmul(
                    ps[:, :], SA[:, zb : zb + TM], WtA[:, :],
                    start=True, stop=False,
                )
                nc.tensor.matmul(
                    ps[:, :], SB[:, zb : zb + TM], WtB[:, :],
                    start=False, stop=True,
                )
                dst = st[:, g * NCH : (g + 1) * NCH]
                if t % 2 == 0:
                    nc.vector.tensor_copy(dst, ps[:, :])
                else:
                    nc.scalar.copy(dst, ps[:, :])
            # one DMA per group: SBUF [p, (g c)] -> DRAM rows [(g p), c],
            # iterated (p, g, c) so the SBUF partition dim stays dim 0
            src3 = st[:, :].rearrange("p (g c) -> p g c", g=GT)
            dst3 = out[g0 * TM : (g0 + GT) * TM, :].rearrange(
                "(g p) c -> p g c", p=TM
            )
            nc.sync.dma_start(dst3, src3)
    nc.compile()
    return nc


_program_cache = {}


def _get_program():
    if "nc" not in _program_cache:
        _program_cache["nc"] = _build_program()
    return _program_cache["nc"]


def _host_weights(atoms_real, atoms_imag, w, w_center, b_center):
    idx = np.repeat(np.arange(DEG + 1), [2 * n + 1 for n in range(DEG + 1)])
    w_exp = w[..., idx]  # [C,F,R,NH]
    WR = np.einsum("dhwrn,cfrn->dhwcfn", atoms_real, w_exp)
    WI = np.einsum("dhwrn,cfrn->dhwcfn", atoms_imag, w_exp)
    Wfull = np.stack([WR, WI], axis=-1)  # [3,3,3,C,F,NH,2]
    Wc = np.zeros((KC, NCH), np.float32)
    Wc[: KC - 1, :] = Wfull.reshape(KC - 1, NCH)
    # central 1x1x1 conv onto (f, n=0, re): tap (kd=1,kh=1,kw=1) rows 104..111
    Wc[104:112, 0::32] += w_center
    Wc[KC - 1, 0::32] = b_center
    return Wc


def kernel(x, atoms_real, atoms_imag, w, w_center, b_center):
    global LAST_RESULTS
    x = np.asarray(x, np.float32)
    Wc = _host_weights(
        np.asarray(atoms_real, np.float32),
        np.asarray(atoms_imag, np.float32),
        np.asarray(w, np.float32),
        np.asarray(w_center, np.float32),
        np.asarray(b_center, np.float32),
    )
    hdt = {"fp16": np.float16, "bf16": ml_dtypes.bfloat16, "f32r": np.float32}[IO_DTYPE]
    Wc = Wc.astype(hdt)

    xt = np.transpose(x[0], (3, 0, 1, 2))  # [C,D,H,W]
    xpad = np.zeros((C, D + 2, HP, WP), np.float32)
    xpad[:, 1 : D + 1, 1 : H + 1, 1 : W + 1] = xt

    in_maps = []
    for core in range(NCORES):
        d0 = core * DL
        pbuf = np.zeros((C, SZ), np.float32)
        pbuf[:, MARGIN : MARGIN + UD] = xpad[:, d0 : d0 + NSLAB].reshape(C, UD)
        buf = np.empty((KC, NZ), np.float32)
        r = 0
        for kd in range(3):
            for kh in range(3):
                for kw in range(3):
                    off = (kd - 1) * SLAB + (kh - 1) * WP + (kw - 1)
                    buf[r : r + 8] = pbuf[:, ZB0 + off : ZB0 + off + NZ]
                    r += 8
        buf[KC - 1] = 1.0
        in_maps.append({"xin": buf.astype(hdt), "wc": Wc})

    nc = _get_program()
    res = run_bass_kernel_spmd(
        nc, in_maps, core_ids=list(range(NCORES)), trace=TRACE
    )
    LAST_RESULTS = res
    outs = [
        res.results[i]["out"][_VALID_IDX].astype(np.float32) for i in range(NCORES)
    ]
    full = np.concatenate([o.reshape(DL, H, W, OUT, NH, 2) for o in outs], axis=0)
    return full[None]


# revision 23
# speedup vs baseline: 3.6764x; 1.1147x over previous
"""Trainium2 Bass kernel for nn_BSHConv3D: spherical-harmonic 3^3 conv.

The whole module collapses to one dense 3D convolution
x[1,48,48,48,8] -> out[48,48,48, 512] with combined weights
W[3,3,3, 8, 512] (the central 1x1x1 conv folds into the center tap, the
bias rides on an extra constant-ones contraction row).

Per-core (D sharded 8 x 6 slabs, halo 1):
  - host builds the FULL 27-tap im2col: S[217, 14976] where row
    (kd,kh,kw,c) is the correspondingly shifted padded x volume and row
    216 is constant ones; z = flattened (d,h,w) padded coords
  - matmul per 128-position tile: 2 PSUM-accumulating matmuls
    (K = 128 + 89 contraction rows) x N=512 output channels
  - PE streams at 1 col / 1.2GHz-cycle here regardless of dtype, so
    2 matmuls/tile (1024 streamed cols) is the floor
  - PSUM evacuated by VectorE/ScalarE alternating into a 9-tile group
    staging buffer, one ~2.3MB output DMA per group (2KB descriptors
    spread across all 16 SDMA engines; bigger merged descriptors pin to
    one engine at ~26GB/s)
  - HBM->SBUF loads ride SWDGE (gpsimd) for the same reason
"""

from contextlib import ExitStack

import ml_dtypes
import numpy as np

import concourse.bass as bass
from concourse import bacc
import concourse.mybir as mybir
import concourse.tile as tile
from concourse.bass_utils import run_bass_kernel_spmd

B, D, H, W, C = 1, 48, 48, 48, 8
KS, R, DEG, NH, OUT = 3, 2, 3, 16, 16
NCORES = 8
DL = D // NCORES  # 6 output slabs per core
HP = WP = 50  # zero-padded H/W
SLAB = HP * WP  # 2500
NSLAB = DL + 2  # local slabs incl. halos
MARGIN = 64  # left margin in the host z buffer (shift slack)
UD = NSLAB * SLAB  # 20000 payload columns
SZ = 20352  # host-side padded z columns
NCH = OUT * NH * 2  # 512 output channels (f, n, re/im)
KC = 27 * C + 1  # 217 contraction rows: 27 taps x 8 ch + ones row
KA = 128  # contraction chunk A (SBUF partition limit)
KB = KC - KA  # 89
ZB0 = MARGIN + SLAB  # first computed z column (host coords)
TM = 128  # positions per matmul tile
NT = 117  # z tiles per core
NZ = NT * TM  # 14976 z columns materialized on chip
NVALID = DL * H * W  # 13824 valid output rows per core
NPAD = NZ  # padded output rows written
NZ_CHUNKS = 3  # im2col load chunking so matmuls can start early
GT = 9  # z tiles grouped per output DMA (117 = 13 groups of 9)

IO_DTYPE = "fp16"  # "fp16" | "bf16" | "f32r" matmul input dtype
OUT_DTYPE = "fp16"  # "fp16" | "f32" output DMA dtype (host upcasts)

# module-level knobs for the test harness (graders just call kernel())
TRACE = False
LAST_RESULTS = None


def _valid_row_index():
    """Indices into the padded [NPAD] output rows that are real outputs,
    in output raster order."""
    u = np.arange(NPAD) + (ZB0 - MARGIN)
    dl = u // SLAB
    hp = (u % SLAB) // WP
    wp = u % WP
    mask = (dl >= 1) & (dl < 1 + DL) & (hp >= 1) & (hp <= H) & (wp >= 1) & (wp <= W)
    idx = np.nonzero(mask)[0]
    assert idx.size == NVALID, idx.size
    return idx


_VALID_IDX = _valid_row_index()


_MDT = {"fp16": mybir.dt.float16, "bf16": mybir.dt.bfloat16, "f32r": mybir.dt.float32r}


def _build_program():
    f32 = mybir.dt.float32
    mdt = _MDT[IO_DTYPE]
    odt = mybir.dt.float16 if OUT_DTYPE == "fp16" else f32
    nc = bacc.Bacc("TRN2", debug=False)
    xin = nc.dram_tensor("xin", [KC, NZ], mdt, kind="ExternalInput").ap()
    wc = nc.dram_tensor("wc", [KC, NCH], mdt, kind="ExternalInput").ap()
    out = nc.dram_tensor("out", [NPAD, NCH], odt, kind="ExternalOutput").ap()

    with tile.TileContext(nc) as tc, ExitStack() as ctx:
        const_pool = ctx.enter_context(tc.tile_pool(name="const", bufs=1))
        stage_pool = ctx.enter_context(tc.tile_pool(name="stage", bufs=2))
        psum_pool = ctx.enter_context(tc.tile_pool(name="psum", bufs=6, space="PSUM"))

        SA = const_pool.tile([KA, NZ], mdt, name="SA")
        SB = const_pool.tile([KB, NZ], mdt, name="SB")
        WtA = const_pool.tile([KA, NCH], mdt, name="WtA")
        WtB = const_pool.tile([KB, NCH], mdt, name="WtB")

        # all HBM->SBUF loads ride SWDGE (gpsimd): the HWDGE path pins a
        # load to a single SDMA engine; SWDGE spreads across all 16
        nc.gpsimd.dma_start(WtA[:, :], wc[0:KA])
        nc.gpsimd.dma_start(WtB[:, :], wc[KA:KC])

        nzc = NZ // NZ_CHUNKS
        for zc in range(NZ_CHUNKS):
            lo, hi = zc * nzc, (zc + 1) * nzc
            for p0, p1 in ((0, 32), (32, 64), (64, 96), (96, 128)):
                nc.gpsimd.dma_start(SA[p0:p1, lo:hi], xin[p0:p1, lo:hi])
            for p0, p1 in ((0, 45), (45, 89)):
                nc.gpsimd.dma_start(
                    SB[p0:p1, lo:hi], xin[KA + p0 : KA + p1, lo:hi]
                )
        del f32  # staging dtype below follows the output dtype

        for g0 in range(0, NT, GT):
            st = stage_pool.tile([TM, GT * NCH], odt, name="st")
            for g in range(GT):
                t = g0 + g
                zb = t * TM
                ps = psum_pool.tile([TM, NCH], mybir.dt.float32, name="ps")
                nc.tensor.matmul(
                    ps[:, :], SA[:, zb : zb + TM], WtA[:, :],
                    start=True, stop=False,
                )
                nc.tensor.matmul(
                    ps[:, :], SB[:, zb : zb + TM], WtB[:, :],
                    start=False, stop=True,
                )
                dst = st[:, g * NCH : (g + 1) * NCH]
                if t % 2 == 0:
                    nc.vector.tensor_copy(dst, ps[:, :])
                else:
                    nc.scalar.copy(dst, ps[:, :])
            # one DMA per group: SBUF [p, (g c)] -> DRAM rows [(g p), c],
            # iterated (p, g, c) so the SBUF partition dim stays dim 0
            src3 = st[:, :].rearrange("p (g c) -> p g c", g=GT)
            dst3 = out[g0 * TM : (g0 + GT) * TM, :].rearrange(
                "(g p) c -> p g c", p=TM
            )
            nc.sync.dma_start(dst3, src3)
    nc.compile()
    return nc


_program_cache = {}


def _get_program():
    if "nc" not in _program_cache:
        _program_cache["nc"] = _build_program()
    return _program_cache["nc"]


def _host_weights(atoms_real, atoms_imag, w, w_center, b_center):
    idx = np.repeat(np.arange(DEG + 1), [2 * n + 1 for n in range(DEG + 1)])
    w_exp = w[..., idx]  # [C,F,R,NH]
    WR = np.einsum("dhwrn,cfrn->dhwcfn", atoms_real, w_exp)
    WI = np.einsum("dhwrn,cfrn->dhwcfn", atoms_imag, w_exp)
    Wfull = np.stack([WR, WI], axis=-1)  # [3,3,3,C,F,NH,2]
    Wc = np.zeros((KC, NCH), np.float32)
    Wc[: KC - 1, :] = Wfull.reshape(KC - 1, NCH)
    # central 1x1x1 conv onto (f, n=0, re): tap (kd=1,kh=1,kw=1) rows 104..111
    Wc[104:112, 0::32] += w_center
    Wc[KC - 1, 0::32] = b_center
    return Wc


def kernel(x, atoms_real, atoms_imag, w, w_center, b_center):
    global LAST_RESULTS
    x = np.asarray(x, np.float32)
    Wc = _host_weights(
        np.asarray(atoms_real, np.float32),
        np.asarray(atoms_imag, np.float32),
        np.asarray(w, np.float32),
        np.asarray(w_center, np.float32),
        np.asarray(b_center, np.float32),
    )
    hdt = {"fp16": np.float16, "bf16": ml_dtypes.bfloat16, "f32r": np.float32}[IO_DTYPE]
    Wc = Wc.astype(hdt)

    xt = np.transpose(x[0], (3, 0, 1, 2))  # [C,D,H,W]
    xpad = np.zeros((C, D + 2, HP, WP), np.float32)
    xpad[:, 1 : D + 1, 1 : H + 1, 1 : W + 1] = xt

    in_maps = []
    for core in range(NCORES):
        d0 = core * DL
        pbuf = np.zeros((C, SZ), np.float32)
        pbuf[:, MARGIN : MARGIN + UD] = xpad[:, d0 : d0 + NSLAB].reshape(C, UD)
        buf = np.empty((KC, NZ), np.float32)
        r = 0
        for kd in range(3):
            for kh in range(3):
                for kw in range(3):
                    off = (kd - 1) * SLAB + (kh - 1) * WP + (kw - 1)
                    buf[r : r + 8] = pbuf[:, ZB0 + off : ZB0 + off + NZ]
                    r += 8
        buf[KC - 1] = 1.0
        in_maps.append({"xin": buf.astype(hdt), "wc": Wc})

    nc = _get_program()
    res = run_bass_kernel_spmd(
        nc, in_maps, core_ids=list(range(NCORES)), trace=TRACE
    )
    LAST_RESULTS = res
    outs = [
        res.results[i]["out"][_VALID_IDX].astype(np.float32) for i in range(NCORES)
    ]
    full = np.concatenate([o.reshape(DL, H, W, OUT, NH, 2) for o in outs], axis=0)
    return full[None]


# revision 27
# speedup vs baseline: 3.9644x; 1.0783x over previous
"""Trainium2 Bass kernel for nn_BSHConv3D: spherical-harmonic 3^3 conv.

The whole module collapses to one dense 3D convolution
x[1,48,48,48,8] -> out[48,48,48, 512] with combined weights
W[3,3,3, 8, 512] (the central 1x1x1 conv folds into the center tap, the
bias rides on an extra constant-ones contraction row).

Per-core (D sharded 8 x 6 slabs, halo 1):
  - host builds the FULL 27-tap im2col: S[217, 14976] where row
    (kd,kh,kw,c) is the correspondingly shifted padded x volume and row
    216 is constant ones; z = flattened (d,h,w) padded coords
  - matmul per 128-position tile: 2 PSUM-accumulating matmuls
    (K = 128 + 89 contraction rows) x N=512 output channels
  - PE streams at 1 col / 1.2GHz-cycle here regardless of dtype, so
    2 matmuls/tile (1024 streamed cols) is the floor
  - PSUM evacuated by VectorE/ScalarE alternating into a 9-tile group
    staging buffer, one ~2.3MB output DMA per group (2KB descriptors
    spread across all 16 SDMA engines; bigger merged descriptors pin to
    one engine at ~26GB/s)
  - HBM->SBUF loads ride SWDGE (gpsimd) for the same reason
"""

from contextlib import ExitStack

import ml_dtypes
import numpy as np

import concourse.bass as bass
from concourse import bacc
import concourse.mybir as mybir
import concourse.tile as tile
from concourse.bass_utils import run_bass_kernel_spmd

B, D, H, W, C = 1, 48, 48, 48, 8
KS, R, DEG, NH, OUT = 3, 2, 3, 16, 16
NCORES = 8
DL = D // NCORES  # 6 output slabs per core
HP = WP = 50  # zero-padded H/W
SLAB = HP * WP  # 2500
NSLAB = DL + 2  # local slabs incl. halos
MARGIN = 64  # left margin in the host z buffer (shift slack)
UD = NSLAB * SLAB  # 20000 payload columns
SZ = 20352  # host-side padded z columns
NCH = OUT * NH * 2  # 512 output channels (f, n, re/im)
KC = 27 * C + 1  # 217 contraction rows: 27 taps x 8 ch + ones row
KA = 128  # contraction chunk A (SBUF partition limit)
KB = KC - KA  # 89
ZB0 = MARGIN + SLAB  # first computed z column (host coords)
TM = 128  # positions per matmul tile
NT = 117  # z tiles per core
NZ = NT * TM  # 14976 z columns materialized on chip
NVALID = DL * H * W  # 13824 valid output rows per core
NPAD = NZ  # padded output rows written
NZ_CHUNKS = 6  # im2col load chunking so matmuls can start early
GT = 9  # z tiles grouped per output DMA (117 = 13 groups of 9)

IO_DTYPE = "fp16"  # "fp16" | "bf16" | "f32r" matmul input dtype
OUT_DTYPE = "fp16"  # "fp16" | "f32" output DMA dtype (host upcasts)

# module-level knobs for the test harness (graders just call kernel())
TRACE = False
LAST_RESULTS = None


def _valid_row_index():
    """Indices into the padded [NPAD] output rows that are real outputs,
    in output raster order."""
    u = np.arange(NPAD) + (ZB0 - MARGIN)
    dl = u // SLAB
    hp = (u % SLAB) // WP
    wp = u % WP
    mask = (dl >= 1) & (dl < 1 + DL) & (hp >= 1) & (hp <= H) & (wp >= 1) & (wp <= W)
    idx = np.nonzero(mask)[0]
    assert idx.size == NVALID, idx.size
    return idx


_VALID_IDX = _valid_row_index()


_MDT = {"fp16": mybir.dt.float16, "bf16": mybir.dt.bfloat16, "f32r": mybir.dt.float32r}


def _build_program():
    f32 = mybir.dt.float32
    mdt = _MDT[IO_DTYPE]
    odt = mybir.dt.float16 if OUT_DTYPE == "fp16" else f32
    nc = bacc.Bacc("TRN2", debug=False)
    xin = nc.dram_tensor("xin", [KC, NZ], mdt, kind="ExternalInput").ap()
    wc = nc.dram_tensor("wc", [KC, NCH], mdt, kind="ExternalInput").ap()
    # output rows permuted [group][p][g][c] so each (partition, group) pair
    # is one contiguous GT*NCH-byte DMA descriptor; host unpermutes
    out = nc.dram_tensor(
        "out", [NT // GT, TM, GT, NCH], odt, kind="ExternalOutput"
    ).ap()

    with tile.TileContext(nc) as tc, ExitStack() as ctx:
        const_pool = ctx.enter_context(tc.tile_pool(name="const", bufs=1))
        stage_pool = ctx.enter_context(tc.tile_pool(name="stage", bufs=2))
        psum_pool = ctx.enter_context(tc.tile_pool(name="psum", bufs=6, space="PSUM"))

        SA = const_pool.tile([KA, NZ], mdt, name="SA")
        SB = const_pool.tile([KB, NZ], mdt, name="SB")
        WtA = const_pool.tile([KA, NCH], mdt, name="WtA")
        WtB = const_pool.tile([KB, NCH], mdt, name="WtB")

        # all HBM->SBUF loads ride SWDGE (gpsimd): the HWDGE path pins a
        # load to a single SDMA engine; SWDGE spreads across all 16
        nc.gpsimd.dma_start(WtA[:, :], wc[0:KA])
        nc.gpsimd.dma_start(WtB[:, :], wc[KA:KC])

        nzc = NZ // NZ_CHUNKS
        for zc in range(NZ_CHUNKS):
            lo, hi = zc * nzc, (zc + 1) * nzc
            for p0, p1 in ((0, 32), (32, 64), (64, 96), (96, 128)):
                nc.gpsimd.dma_start(SA[p0:p1, lo:hi], xin[p0:p1, lo:hi])
            for p0, p1 in ((0, 45), (45, 89)):
                nc.gpsimd.dma_start(
                    SB[p0:p1, lo:hi], xin[KA + p0 : KA + p1, lo:hi]
                )
        del f32  # staging dtype below follows the output dtype

        for g0 in range(0, NT, GT):
            st = stage_pool.tile([TM, GT * NCH], odt, name="st")
            for g in range(GT):
                t = g0 + g
                zb = t * TM
                ps = psum_pool.tile([TM, NCH], mybir.dt.float32, name="ps")
                nc.tensor.matmul(
                    ps[:, :], SA[:, zb : zb + TM], WtA[:, :],
                    start=True, stop=False,
                )
                nc.tensor.matmul(
                    ps[:, :], SB[:, zb : zb + TM], WtB[:, :],
                    start=False, stop=True,
                )
                dst = st[:, g * NCH : (g + 1) * NCH]
                if t % 2 == 0:
                    nc.vector.tensor_copy(dst, ps[:, :])
                else:
                    nc.scalar.copy(dst, ps[:, :])
            # one DMA per group, both sides contiguous per partition
            nc.sync.dma_start(out[g0 // GT], st[:, :])
    nc.compile()
    return nc


_program_cache = {}


def _get_program():
    if "nc" not in _program_cache:
        _program_cache["nc"] = _build_program()
    return _program_cache["nc"]


def _host_weights(atoms_real, atoms_imag, w, w_center, b_center):
    idx = np.repeat(np.arange(DEG + 1), [2 * n + 1 for n in range(DEG + 1)])
    w_exp = w[..., idx]  # [C,F,R,NH]
    WR = np.einsum("dhwrn,cfrn->dhwcfn", atoms_real, w_exp)
    WI = np.einsum("dhwrn,cfrn->dhwcfn", atoms_imag, w_exp)
    Wfull = np.stack([WR, WI], axis=-1)  # [3,3,3,C,F,NH,2]
    Wc = np.zeros((KC, NCH), np.float32)
    Wc[: KC - 1, :] = Wfull.reshape(KC - 1, NCH)
    # central 1x1x1 conv onto (f, n=0, re): tap (kd=1,kh=1,kw=1) rows 104..111
    Wc[104:112, 0::32] += w_center
    Wc[KC - 1, 0::32] = b_center
    return Wc


def kernel(x, atoms_real, atoms_imag, w, w_center, b_center):
    global LAST_RESULTS
    x = np.asarray(x, np.float32)
    Wc = _host_weights(
        np.asarray(atoms_real, np.float32),
        np.asarray(atoms_imag, np.float32),
        np.asarray(w, np.float32),
        np.asarray(w_center, np.float32),
        np.asarray(b_center, np.float32),
    )
    hdt = {"fp16": np.float16, "bf16": ml_dtypes.bfloat16, "f32r": np.float32}[IO_DTYPE]
    Wc = Wc.astype(hdt)

    xt = np.transpose(x[0], (3, 0, 1, 2))  # [C,D,H,W]
    xpad = np.zeros((C, D + 2, HP, WP), np.float32)
    xpad[:, 1 : D + 1, 1 : H + 1, 1 : W + 1] = xt

    in_maps = []
    for core in range(NCORES):
        d0 = core * DL
        pbuf = np.zeros((C, SZ), np.float32)
        pbuf[:, MARGIN : MARGIN + UD] = xpad[:, d0 : d0 + NSLAB].reshape(C, UD)
        buf = np.empty((KC, NZ), np.float32)
        r = 0
        for kd in range(3):
            for kh in range(3):
                for kw in range(3):
                    off = (kd - 1) * SLAB + (kh - 1) * WP + (kw - 1)
                    buf[r : r + 8] = pbuf[:, ZB0 + off : ZB0 + off + NZ]
                    r += 8
        buf[KC - 1] = 1.0
        in_maps.append({"xin": buf.astype(hdt), "wc": Wc})

    nc = _get_program()
    res = run_bass_kernel_spmd(
        nc, in_maps, core_ids=list(range(NCORES)), trace=TRACE
    )
    LAST_RESULTS = res
    outs = [
        res.results[i]["out"]
        .transpose(0, 2, 1, 3)
        .reshape(NPAD, NCH)[_VALID_IDX]
        .astype(np.float32)
        for i in range(NCORES)
    ]
    full = np.concatenate([o.reshape(DL, H, W, OUT, NH, 2) for o in outs], axis=0)
    return full[None]


# revision 30
# speedup vs baseline: 4.0950x; 1.0330x over previous
"""Trainium2 Bass kernel for nn_BSHConv3D: spherical-harmonic 3^3 conv.

The whole module collapses to one dense 3D convolution
x[1,48,48,48,8] -> out[48,48,48, 512] with combined weights
W[3,3,3, 8, 512] (the central 1x1x1 conv folds into the center tap, the
bias rides on an extra constant-ones contraction row).

Per-core (D sharded 8 x 6 slabs, halo 1):
  - host builds the FULL 27-tap im2col: S[217, 14976] where row
    (kd,kh,kw,c) is the correspondingly shifted padded x volume and row
    216 is constant ones; z = flattened (d,h,w) padded coords
  - matmul per 128-position tile: 2 PSUM-accumulating matmuls
    (K = 128 + 89 contraction rows) x N=512 output channels
  - PE streams at 1 col / 1.2GHz-cycle here regardless of dtype, so
    2 matmuls/tile (1024 streamed cols) is the floor
  - PSUM evacuated by VectorE/ScalarE alternating into a 9-tile group
    staging buffer, one ~2.3MB output DMA per group (2KB descriptors
    spread across all 16 SDMA engines; bigger merged descriptors pin to
    one engine at ~26GB/s)
  - HBM->SBUF loads ride SWDGE (gpsimd) for the same reason
"""

from contextlib import ExitStack

import ml_dtypes
import numpy as np

import concourse.bass as bass
from concourse import bacc
import concourse.mybir as mybir
import concourse.tile as tile
from concourse.bass_utils import run_bass_kernel_spmd

B, D, H, W, C = 1, 48, 48, 48, 8
KS, R, DEG, NH, OUT = 3, 2, 3, 16, 16
NCORES = 8
DL = D // NCORES  # 6 output slabs per core
HP = WP = 50  # zero-padded H/W
SLAB = HP * WP  # 2500
NSLAB = DL + 2  # local slabs incl. halos
MARGIN = 64  # left margin in the host z buffer (shift slack)
UD = NSLAB * SLAB  # 20000 payload columns
SZ = 20352  # host-side padded z columns
NCH = OUT * NH * 2  # 512 output channels (f, n, re/im)
KC = 27 * C + 1  # 217 contraction rows: 27 taps x 8 ch + ones row
KA = 128  # contraction chunk A (SBUF partition limit)
KB = KC - KA  # 89
ZB0 = MARGIN + SLAB  # first computed z column (host coords)
TM = 128  # positions per matmul tile
NT = 117  # z tiles per core
NZ = NT * TM  # 14976 z columns materialized on chip
NVALID = DL * H * W  # 13824 valid output rows per core
NPAD = NZ  # padded output rows written
NZ_CHUNKS = 6  # im2col load chunking so matmuls can start early
GT = 9  # z tiles grouped per output DMA (117 = 13 groups of 9)

IO_DTYPE = "fp16"  # "fp16" | "bf16" | "f32r" matmul input dtype
OUT_DTYPE = "fp16"  # "fp16" | "f32" output DMA dtype (host upcasts)

# module-level knobs for the test harness (graders just call kernel())
TRACE = False
LAST_RESULTS = None


def _valid_row_index():
    """Indices into the padded [NPAD] output rows that are real outputs,
    in output raster order."""
    u = np.arange(NPAD) + (ZB0 - MARGIN)
    dl = u // SLAB
    hp = (u % SLAB) // WP
    wp = u % WP
    mask = (dl >= 1) & (dl < 1 + DL) & (hp >= 1) & (hp <= H) & (wp >= 1) & (wp <= W)
    idx = np.nonzero(mask)[0]
    assert idx.size == NVALID, idx.size
    return idx


_VALID_IDX = _valid_row_index()


_MDT = {"fp16": mybir.dt.float16, "bf16": mybir.dt.bfloat16, "f32r": mybir.dt.float32r}


def _build_program():
    f32 = mybir.dt.float32
    mdt = _MDT[IO_DTYPE]
    odt = mybir.dt.float16 if OUT_DTYPE == "fp16" else f32
    nc = bacc.Bacc("TRN2", debug=False)
    xin = nc.dram_tensor("xin", [KC, NZ], mdt, kind="ExternalInput").ap()
    wc = nc.dram_tensor("wc", [KC, NCH], mdt, kind="ExternalInput").ap()
    # output rows permuted [group][p][g][c] so each (partition, group) pair
    # is one contiguous GT*NCH-byte DMA descriptor; host unpermutes
    out = nc.dram_tensor(
        "out", [NT // GT, TM, GT, NCH], odt, kind="ExternalOutput"
    ).ap()

    with tile.TileContext(nc) as tc, ExitStack() as ctx:
        const_pool = ctx.enter_context(tc.tile_pool(name="const", bufs=1))
        stage_pool = ctx.enter_context(tc.tile_pool(name="stage", bufs=2))
        psum_pool = ctx.enter_context(tc.tile_pool(name="psum", bufs=8, space="PSUM"))

        SA = const_pool.tile([KA, NZ], mdt, name="SA")
        SB = const_pool.tile([KB, NZ], mdt, name="SB")
        WtA = const_pool.tile([KA, NCH], mdt, name="WtA")
        WtB = const_pool.tile([KB, NCH], mdt, name="WtB")

        # all HBM->SBUF loads ride SWDGE (gpsimd): the HWDGE path pins a
        # load to a single SDMA engine; SWDGE spreads across all 16
        nc.gpsimd.dma_start(WtA[:, :], wc[0:KA])
        nc.gpsimd.dma_start(WtB[:, :], wc[KA:KC])

        # non-uniform chunks: small early chunks start the matmuls fast,
        # big later chunks amortize per-descriptor overhead
        lo = 0
        for frac in (12, 12, 6, 6, 4, 4):
            hi = min(NZ, lo + NZ // frac)
            for p0, p1 in ((0, 32), (32, 64), (64, 96), (96, 128)):
                nc.gpsimd.dma_start(SA[p0:p1, lo:hi], xin[p0:p1, lo:hi])
            for p0, p1 in ((0, 45), (45, 89)):
                nc.gpsimd.dma_start(
                    SB[p0:p1, lo:hi], xin[KA + p0 : KA + p1, lo:hi]
                )
            lo = hi
        assert lo == NZ, lo
        del f32  # staging dtype below follows the output dtype

        for g0 in range(0, NT, GT):
            st = stage_pool.tile([TM, GT * NCH], odt, name="st")
            for g in range(GT):
                t = g0 + g
                zb = t * TM
                ps = psum_pool.tile([TM, NCH], mybir.dt.float32, name="ps")
                nc.tensor.matmul(
                    ps[:, :], SA[:, zb : zb + TM], WtA[:, :],
                    start=True, stop=False,
                )
                nc.tensor.matmul(
                    ps[:, :], SB[:, zb : zb + TM], WtB[:, :],
                    start=False, stop=True,
                )
                dst = st[:, g * NCH : (g + 1) * NCH]
                if t % 2 == 0:
                    nc.vector.tensor_copy(dst, ps[:, :])
                else:
                    nc.scalar.copy(dst, ps[:, :])
            # one DMA per group, both sides contiguous per partition; the
            # last group drains in 3-tile sub-DMAs to shorten the tail
            if g0 + GT < NT:
                nc.sync.dma_start(out[g0 // GT], st[:, :])
            else:
                for s in range(0, GT, 3):
                    nc.sync.dma_start(
                        out[g0 // GT][:, s : s + 3, :],
                        st[:, s * NCH : (s + 3) * NCH],
                    )
    nc.compile()
    return nc


_program_cache = {}


def _get_program():
    if "nc" not in _program_cache:
        _program_cache["nc"] = _build_program()
    return _program_cache["nc"]


def _host_weights(atoms_real, atoms_imag, w, w_center, b_center):
    idx = np.repeat(np.arange(DEG + 1), [2 * n + 1 for n in range(DEG + 1)])
    w_exp = w[..., idx]  # [C,F,R,NH]
    WR = np.einsum("dhwrn,cfrn->dhwcfn", atoms_real, w_exp)
    WI = np.einsum("dhwrn,cfrn->dhwcfn", atoms_imag, w_exp)
    Wfull = np.stack([WR, WI], axis=-1)  # [3,3,3,C,F,NH,2]
    Wc = np.zeros((KC, NCH), np.float32)
    Wc[: KC - 1, :] = Wfull.reshape(KC - 1, NCH)
    # central 1x1x1 conv onto (f, n=0, re): tap (kd=1,kh=1,kw=1) rows 104..111
    Wc[104:112, 0::32] += w_center
    Wc[KC - 1, 0::32] = b_center
    return Wc


def kernel(x, atoms_real, atoms_imag, w, w_center, b_center):
    global LAST_RESULTS
    x = np.asarray(x, np.float32)
    Wc = _host_weights(
        np.asarray(atoms_real, np.float32),
        np.asarray(atoms_imag, np.float32),
        np.asarray(w, np.float32),
        np.asarray(w_center, np.float32),
        np.asarray(b_center, np.float32),
    )
    hdt = {"fp16": np.float16, "bf16": ml_dtypes.bfloat16, "f32r": np.float32}[IO_DTYPE]
    Wc = Wc.astype(hdt)

    xt = np.transpose(x[0], (3, 0, 1, 2))  # [C,D,H,W]
    xpad = np.zeros((C, D + 2, HP, WP), np.float32)
    xpad[:, 1 : D + 1, 1 : H + 1, 1 : W + 1] = xt

    in_maps = []
    for core in range(NCORES):
        d0 = core * DL
        pbuf = np.zeros((C, SZ), np.float32)
        pbuf[:, MARGIN : MARGIN + UD] = xpad[:, d0 : d0 + NSLAB].reshape(C, UD)
        buf = np.empty((KC, NZ), np.float32)
        r = 0
        for kd in range(3):
            for kh in range(3):
                for kw in range(3):
                    off = (kd - 1) * SLAB + (kh - 1) * WP + (kw - 1)
                    buf[r : r + 8] = pbuf[:, ZB0 + off : ZB0 + off + NZ]
                    r += 8
        buf[KC - 1] = 1.0
        in_maps.append({"xin": buf.astype(hdt), "wc": Wc})

    nc = _get_program()
    res = run_bass_kernel_spmd(
        nc, in_maps, core_ids=list(range(NCORES)), trace=TRACE
    )
    LAST_RESULTS = res
    outs = [
        res.results[i]["out"]
        .transpose(0, 2, 1, 3)
        .reshape(NPAD, NCH)[_VALID_IDX]
        .astype(np.float32)
        for i in range(NCORES)
    ]
    full = np.concatenate([o.reshape(DL, H, W, OUT, NH, 2) for o in outs], axis=0)
    return full[None]


# revision 31
# speedup vs baseline: 4.6836x; 1.1437x over previous
"""Trainium2 Bass kernel for nn_BSHConv3D: spherical-harmonic 3^3 conv.

The whole module collapses to one dense 3D convolution
x[1,48,48,48,8] -> out[48,48,48, 512] with combined weights
W[3,3,3, 8, 512] (the central 1x1x1 conv folds into the center tap, the
bias rides on an extra constant-ones contraction row).

Per-core (D sharded 8 x 6 slabs, halo 1):
  - host builds the FULL 27-tap im2col: S[217, 14976] where row
    (kd,kh,kw,c) is the correspondingly shifted padded x volume and row
    216 is constant ones; z = flattened (d,h,w) padded coords
  - matmul per 128-position tile: 2 PSUM-accumulating matmuls
    (K = 128 + 89 contraction rows) x N=512 output channels
  - PE streams at 1 col / 1.2GHz-cycle here regardless of dtype, so
    2 matmuls/tile (1024 streamed cols) is the floor
  - PSUM evacuated by VectorE/ScalarE alternating into a 9-tile group
    staging buffer, one ~2.3MB output DMA per group (2KB descriptors
    spread across all 16 SDMA engines; bigger merged descriptors pin to
    one engine at ~26GB/s)
  - HBM->SBUF loads ride SWDGE (gpsimd) for the same reason
"""

from contextlib import ExitStack

import ml_dtypes
import numpy as np

import concourse.bass as bass
from concourse import bacc
import concourse.mybir as mybir
import concourse.tile as tile
from concourse.bass_utils import run_bass_kernel_spmd

B, D, H, W, C = 1, 48, 48, 48, 8
KS, R, DEG, NH, OUT = 3, 2, 3, 16, 16
NCORES = 8
DL = D // NCORES  # 6 output slabs per core
HP = WP = 50  # zero-padded H/W
SLAB = HP * WP  # 2500
NSLAB = DL + 2  # local slabs incl. halos
MARGIN = 64  # left margin in the host z buffer (shift slack)
UD = NSLAB * SLAB  # 20000 payload columns
SZ = 20352  # host-side padded z columns
NCH = OUT * NH * 2  # 512 output channels (f, n, re/im)
KC = 27 * C + 1  # 217 contraction rows: 27 taps x 8 ch + ones row
KA = 128  # contraction chunk A (SBUF partition limit)
KB = KC - KA  # 89
ZB0 = MARGIN + SLAB  # first computed z column (host coords)
TM = 128  # positions per matmul tile
NT = 117  # z tiles per core
NZ = NT * TM  # 14976 z columns materialized on chip
NVALID = DL * H * W  # 13824 valid output rows per core
NPAD = NZ  # padded output rows written
NZ_CHUNKS = 6  # im2col load chunking so matmuls can start early
GT = 9  # z tiles grouped per output DMA (117 = 13 groups of 9)

IO_DTYPE = "fp16"  # "fp16" | "bf16" | "f32r" matmul input dtype
OUT_DTYPE = "fp16"  # "fp16" | "f32" output DMA dtype (host upcasts)

# module-level knobs for the test harness (graders just call kernel())
TRACE = False
LAST_RESULTS = None


def _valid_row_index():
    """Indices into the padded [NPAD] output rows that are real outputs,
    in output raster order."""
    u = np.arange(NPAD) + (ZB0 - MARGIN)
    dl = u // SLAB
    hp = (u % SLAB) // WP
    wp = u % WP
    mask = (dl >= 1) & (dl < 1 + DL) & (hp >= 1) & (hp <= H) & (wp >= 1) & (wp <= W)
    idx = np.nonzero(mask)[0]
    assert idx.size == NVALID, idx.size
    return idx


_VALID_IDX = _valid_row_index()


_MDT = {"fp16": mybir.dt.float16, "bf16": mybir.dt.bfloat16, "f32r": mybir.dt.float32r}


def _build_program():
    f32 = mybir.dt.float32
    mdt = _MDT[IO_DTYPE]
    odt = mybir.dt.float16 if OUT_DTYPE == "fp16" else f32
    nc = bacc.Bacc("TRN2", debug=False)
    xin = nc.dram_tensor("xin", [KC, NZ], mdt, kind="ExternalInput").ap()
    wc = nc.dram_tensor("wc", [KC, NCH], mdt, kind="ExternalInput").ap()
    # output rows permuted [group][p][g][c] so each (partition, group) pair
    # is one contiguous GT*NCH-byte DMA descriptor; host unpermutes
    out = nc.dram_tensor(
        "out", [NT // GT, TM, GT, NCH], odt, kind="ExternalOutput"
    ).ap()

    with tile.TileContext(nc) as tc, ExitStack() as ctx:
        const_pool = ctx.enter_context(tc.tile_pool(name="const", bufs=1))
        stage_pool = ctx.enter_context(tc.tile_pool(name="stage", bufs=3))
        psum_pool = ctx.enter_context(tc.tile_pool(name="psum", bufs=8, space="PSUM"))

        SA = const_pool.tile([KA, NZ], mdt, name="SA")
        SB = const_pool.tile([KB, NZ], mdt, name="SB")
        WtA = const_pool.tile([KA, NCH], mdt, name="WtA")
        WtB = const_pool.tile([KB, NCH], mdt, name="WtB")

        # all HBM->SBUF loads ride SWDGE (gpsimd): the HWDGE path pins a
        # load to a single SDMA engine; SWDGE spreads across all 16
        nc.gpsimd.dma_start(WtA[:, :], wc[0:KA])
        nc.gpsimd.dma_start(WtB[:, :], wc[KA:KC])

        # non-uniform chunks: small early chunks start the matmuls fast,
        # big later chunks amortize per-descriptor overhead
        lo = 0
        for frac in (16, 16, 8, 8, 8, 4, 4):
            hi = min(NZ, lo + NZ // frac)
            for p0, p1 in ((0, 32), (32, 64), (64, 96), (96, 128)):
                nc.gpsimd.dma_start(SA[p0:p1, lo:hi], xin[p0:p1, lo:hi])
            for p0, p1 in ((0, 45), (45, 89)):
                nc.gpsimd.dma_start(
                    SB[p0:p1, lo:hi], xin[KA + p0 : KA + p1, lo:hi]
                )
            lo = hi
        assert lo == NZ, lo
        del f32  # staging dtype below follows the output dtype

        for g0 in range(0, NT, GT):
            st = stage_pool.tile([TM, GT * NCH], odt, name="st")
            for g in range(GT):
                t = g0 + g
                zb = t * TM
                ps = psum_pool.tile([TM, NCH], mybir.dt.float32, name="ps")
                nc.tensor.matmul(
                    ps[:, :], SA[:, zb : zb + TM], WtA[:, :],
                    start=True, stop=False,
                )
                nc.tensor.matmul(
                    ps[:, :], SB[:, zb : zb + TM], WtB[:, :],
                    start=False, stop=True,
                )
                dst = st[:, g * NCH : (g + 1) * NCH]
                if t % 2 == 0:
                    nc.vector.tensor_copy(dst, ps[:, :])
                else:
                    nc.scalar.copy(dst, ps[:, :])
            # one DMA per group, both sides contiguous per partition; the
            # last group drains in 3-tile sub-DMAs to shorten the tail
            if g0 + GT < NT:
                nc.sync.dma_start(out[g0 // GT], st[:, :])
            else:
                for s in range(0, GT, 3):
                    nc.sync.dma_start(
                        out[g0 // GT][:, s : s + 3, :],
                        st[:, s * NCH : (s + 3) * NCH],
                    )
    nc.compile()
    return nc


_program_cache = {}


def _get_program():
    if "nc" not in _program_cache:
        _program_cache["nc"] = _build_program()
    return _program_cache["nc"]


def _host_weights(atoms_real, atoms_imag, w, w_center, b_center):
    idx = np.repeat(np.arange(DEG + 1), [2 * n + 1 for n in range(DEG + 1)])
    w_exp = w[..., idx]  # [C,F,R,NH]
    WR = np.einsum("dhwrn,cfrn->dhwcfn", atoms_real, w_exp)
    WI = np.einsum("dhwrn,cfrn->dhwcfn", atoms_imag, w_exp)
    Wfull = np.stack([WR, WI], axis=-1)  # [3,3,3,C,F,NH,2]
    Wc = np.zeros((KC, NCH), np.float32)
    Wc[: KC - 1, :] = Wfull.reshape(KC - 1, NCH)
    # central 1x1x1 conv onto (f, n=0, re): tap (kd=1,kh=1,kw=1) rows 104..111
    Wc[104:112, 0::32] += w_center
    Wc[KC - 1, 0::32] = b_center
    return Wc


def kernel(x, atoms_real, atoms_imag, w, w_center, b_center):
    global LAST_RESULTS
    x = np.asarray(x, np.float32)
    Wc = _host_weights(
        np.asarray(atoms_real, np.float32),
        np.asarray(atoms_imag, np.float32),
        np.asarray(w, np.float32),
        np.asarray(w_center, np.float32),
        np.asarray(b_center, np.float32),
    )
    hdt = {"fp16": np.float16, "bf16": ml_dtypes.bfloat16, "f32r": np.float32}[IO_DTYPE]
    Wc = Wc.astype(hdt)

    xt = np.transpose(x[0], (3, 0, 1, 2))  # [C,D,H,W]
    xpad = np.zeros((C, D + 2, HP, WP), np.float32)
    xpad[:, 1 : D + 1, 1 : H + 1, 1 : W + 1] = xt

    in_maps = []
    for core in range(NCORES):
        d0 = core * DL
        pbuf = np.zeros((C, SZ), np.float32)
        pbuf[:, MARGIN : MARGIN + UD] = xpad[:, d0 : d0 + NSLAB].reshape(C, UD)
        buf = np.empty((KC, NZ), np.float32)
        r = 0
        for kd in range(3):
            for kh in range(3):
                for kw in range(3):
                    off = (kd - 1) * SLAB + (kh - 1) * WP + (kw - 1)
                    buf[r : r + 8] = pbuf[:, ZB0 + off : ZB0 + off + NZ]
                    r += 8
        buf[KC - 1] = 1.0
        in_maps.append({"xin": buf.astype(hdt), "wc": Wc})

    nc = _get_program()
    res = run_bass_kernel_spmd(
        nc, in_maps, core_ids=list(range(NCORES)), trace=TRACE
    )
    LAST_RESULTS = res
    outs = [
        res.results[i]["out"]
        .transpose(0, 2, 1, 3)
        .reshape(NPAD, NCH)[_VALID_IDX]
        .astype(np.float32)
        for i in range(NCORES)
    ]
    full = np.concatenate([o.reshape(DL, H, W, OUT, NH, 2) for o in outs], axis=0)
    return full[None]
